# revision 17
# baseline (speedup 1.0000x reference)
"""Trainium2 Bass kernel: bidirectional-LSTM final-cell-state encoder.

Problem: 4 independent BasicLSTMCell chains (premise fw/bw, hypothesis fw/bw),
B=1024, T=128, D=300, H=100.  Output [B, 4H] = concat of final cell states.

Strategy
--------
* Data-parallel: batch sharded 8 ways -> 128 rows/core, each core runs all
  4 chains (as 2 decoupled pair-chains) so recurrences hide each other's
  latency.
* Natural layout: z_t [128b, 400g] accumulated in PSUM from 4 matmuls
  (3 pre-transposed x chunks in fp32r + recurrent h^T chunk in bf16).
* x is pre-transposed on the host to [T, 303, B_local] (d-major) so the
  stationary matmul operand streams straight from DRAM; a ones-row is baked
  in at d-row 100 of each 101-row chunk so bias (and the +1.0 forget bias)
  fold into the matmul for free (W row 100 of chunk 2 carries them).
* Gate columns permuted host-side to (i, f, o, j) so ACT does
  sigmoid(i,f,o) in ONE strided instruction + tanh(j), per run-pair
  (batched across 2 runs each).
* h^T for the next step: bf16 PE transpose into dead PSUM columns of the z
  tile, evacuated by one DVE copy per pair.
"""

import numpy as np

B, T, D, H = 1024, 128, 300, 100
NCORES = 8
BL = B // NCORES          # 128 batch rows per core
G4 = 4 * H                # 400 gate columns
KCH = 128                 # 100 d-rows + 1 ones-row + zero-pad (FWL needs K=128)
TB = 8                    # timesteps per DMA block
FORGET_BIAS = 1.0

_CACHE = {}


def _build_program(n_steps=T):
    from contextlib import ExitStack

    import concourse.mybir as mybir
    import concourse.tile as tile
    from concourse import bacc

    f32 = mybir.dt.float32
    f32r = mybir.dt.float32r
    bf16 = mybir.dt.bfloat16
    Sig = mybir.ActivationFunctionType.Sigmoid
    Tanh = mybir.ActivationFunctionType.Tanh
    mult = mybir.AluOpType.mult
    add = mybir.AluOpType.add

    nc = bacc.Bacc(
        "TRN2",
        target_bir_lowering=False,
        debug=False,
        enable_asserts=False,
        num_devices=NCORES,
    )

    xt_p = nc.dram_tensor("xt_p", [T // TB, KCH, TB * 3 * BL], bf16, kind="ExternalInput").ap()
    xt_h = nc.dram_tensor("xt_h", [T // TB, KCH, TB * 3 * BL], bf16, kind="ExternalInput").ap()
    w_all = nc.dram_tensor("w_all", [KCH, 16 * G4], bf16, kind="ExternalInput").ap()
    wh_bf = nc.dram_tensor("wh_bf", [128, 4 * G4], bf16, kind="ExternalInput").ap()
    ident = nc.dram_tensor("ident", [128, 128], bf16, kind="ExternalInput").ap()
    out = nc.dram_tensor("out", [BL, G4], f32, kind="ExternalOutput").ap()

    with tile.TileContext(nc) as tc, ExitStack() as ctx:
        w_sb = nc.alloc_sbuf_tensor("w_sb", [KCH, 16 * G4], bf16).ap()
        wh_sb = nc.alloc_sbuf_tensor("wh_sb", [128, 4 * G4], bf16).ap()
        id_sb = nc.alloc_sbuf_tensor("id_sb", [128, 128], bf16).ap()

        # per-pair state/intermediate tensors (pair p owns runs 2p, 2p+1)
        cP, SP, TJP, TCP, T1P, T2P, HNP, HTP = [], [], [], [], [], [], [], []
        for p in range(2):
            cP.append(nc.alloc_sbuf_tensor(f"c{p}", [BL, 200], f32).ap())
            SP.append(nc.alloc_sbuf_tensor(f"s{p}", [BL, 600], bf16).ap())
            TJP.append(nc.alloc_sbuf_tensor(f"tj{p}", [BL, 200], bf16).ap())
            TCP.append(nc.alloc_sbuf_tensor(f"tc{p}", [BL, 200], bf16).ap())
            T1P.append(nc.alloc_sbuf_tensor(f"t1{p}", [BL, 200], bf16).ap())
            T2P.append(nc.alloc_sbuf_tensor(f"t2{p}", [BL, 200], f32).ap())
            HNP.append(nc.alloc_sbuf_tensor(f"hn{p}", [BL, 200], bf16).ap())
            HTP.append(nc.alloc_sbuf_tensor(f"ht{p}", [128, 256], bf16).ap())

        nc.gpsimd.dma_start(w_sb, w_all)
        nc.gpsimd.dma_start(wh_sb, wh_bf)
        nc.gpsimd.dma_start(id_sb, ident)
        for p in range(2):
            nc.vector.memset(cP[p], 0.0)
            nc.vector.memset(HTP[p], 0.0)

        xt_pools = [
            ctx.enter_context(tc.tile_pool(name=f"xt{s}", bufs=2)) for s in range(4)
        ]
        zpools = [
            ctx.enter_context(tc.tile_pool(name=f"zp{p}", bufs=2, space="PSUM"))
            for p in range(2)
        ]

        # stream s: (dram tensor, reversed?) for runs (p_fw, p_bw, h_fw, h_bw)
        streams = [(xt_p, False), (xt_p, True), (xt_h, False), (xt_h, True)]
        cur = [None] * 4
        pend = None

        for tt in range(n_steps):
            t = tt % T
            if t % TB == 0:
                for s, (dram, rev) in enumerate(streams):
                    tl = xt_pools[s].tile(
                        [KCH, TB * 3 * 128], bf16, tag=f"x{s}", name=f"x{s}_{tt}"
                    )
                    nblk = ((T - TB - t) if rev else t) // TB
                    nc.gpsimd.dma_start(tl[:, :], dram[nblk])
                    cur[s] = tl

            z = [
                zpools[p].tile([BL, 1024], f32, tag=f"z{p}", name=f"z{p}_{tt}")
                for p in range(2)
            ]
            # x-projection matmuls first: independent of h(t-1), so they
            # fill the PE FIFO while the previous step's gate math runs
            for u in range(4):
                p, r = divmod(u, 2)
                rev = streams[u][1]
                tq = (TB - 1 - t % TB) if rev else (t % TB)
                zc = z[p][:, r * 512 : r * 512 + G4]
                tl = cur[u]
                for k in range(3):
                    nc.tensor.matmul(
                        zc,
                        tl[:, (tq * 3 + k) * 128 : (tq * 3 + k + 1) * 128],
                        w_sb[:, (u * 4 + k) * G4 : (u * 4 + k + 1) * G4],
                        start=(k == 0),
                        stop=False,
                    )
            # previous step's h transposes + evac sit between the x-MMs and
            # the recurrent MMs that consume them
            if pend is not None:
                for p in range(2):
                    zo = pend[p]
                    for r in range(2):
                        nc.tensor.transpose(
                            zo[0:H, r * 512 : r * 512 + 64].bitcast(bf16),
                            HNP[p][:, r * 100 : r * 100 + 100],
                            id_sb,
                        )
                    hsrc = (
                        zo[0:H, :]
                        .bitcast(bf16)
                        .rearrange("q (r c) -> q r c", r=2)[:, :, 0:128]
                    )
                    hdst = HTP[p][0:H, :].rearrange("q (r c) -> q r c", r=2)
                    nc.vector.tensor_copy(hdst, hsrc)
            for u in range(4):
                p, r = divmod(u, 2)
                zc = z[p][:, r * 512 : r * 512 + G4]
                nc.tensor.matmul(
                    zc,
                    HTP[p][:, r * 128 : (r + 1) * 128],
                    wh_sb[:, u * G4 : (u + 1) * G4],
                    start=False,
                    stop=True,
                )

            for p in range(2):
                zp = z[p]
                z3 = zp[:, :].rearrange("b (r c) -> b r c", r=2)
                s3 = SP[p][:, :].rearrange("b (r c) -> b r c", r=2)   # [BL,2,300]
                tj3 = TJP[p][:, :].rearrange("b (r c) -> b r c", r=2)  # [BL,2,100]
                tc3 = TCP[p][:, :].rearrange("b (r c) -> b r c", r=2)
                c3 = cP[p][:, :].rearrange("b (r c) -> b r c", r=2)
                t13 = T1P[p][:, :].rearrange("b (r c) -> b r c", r=2)
                t23 = T2P[p][:, :].rearrange("b (r c) -> b r c", r=2)
                hn3 = HNP[p][:, :].rearrange("b (r c) -> b r c", r=2)

                nc.scalar.activation(s3, z3[:, :, 0:300], Sig)        # i, f, o
                nc.scalar.activation(tj3, z3[:, :, 300:400], Tanh)   # j

                nc.vector.tensor_tensor(t13, s3[:, :, 0:100], tj3, mult)
                nc.vector.tensor_tensor(t23, s3[:, :, 100:200], c3, mult)
                nc.vector.tensor_tensor(cP[p], T1P[p], T2P[p], add)
                nc.scalar.activation(TCP[p], cP[p], Tanh)
                nc.vector.tensor_tensor(hn3, tc3, s3[:, :, 200:300], mult)

            pend = z
            if tt == n_steps - 1:
                for p in range(2):
                    nc.sync.dma_start(out[:, p * 200 : (p + 1) * 200], cP[p])

    nc.compile()
    return nc


def _prep_xt(x_slice):
    """[BL, T, D] fp32 -> [T//TB, 101, TB*3*BL] bf16 block-major tiles.

    tile[n, p, (tq, j, b)] = x[b, n*TB+tq, j*100+p] for p<100; p=100 is the
    baked-in ones row (bias trick).  Each DMA block is a plain 2D copy with
    TB*3*BL*2 contiguous bytes per partition.
    """
    import ml_dtypes

    a = x_slice.transpose(1, 2, 0).reshape(T // TB, TB, 3, 100, BL)
    a = a.transpose(0, 3, 1, 2, 4)  # [n, p, tq, j, b]
    outp = np.zeros((T // TB, KCH, TB, 3, BL), ml_dtypes.bfloat16)
    outp[:, :100] = a.astype(ml_dtypes.bfloat16)
    outp[:, 100] = 1.0
    return outp.reshape(T // TB, KCH, TB * 3 * BL)


def _prep_weights(Ws, bs):
    """Pack 4 runs' [D+H, 4H] weights into [101, 16*400] chunk blocks.

    Gate columns permuted (i,j,f,o) -> (i,f,o,j); chunk-2's row 100 carries
    b_perm + the +1.0 forget bias (paired with the baked-in x ones-row).
    Also emits the recurrent rows (300:400) as bf16 [100, 4*400].
    """
    import ml_dtypes

    perm = np.concatenate(
        [np.arange(0, 100), np.arange(200, 300), np.arange(300, 400), np.arange(100, 200)]
    )
    w_all = np.zeros((KCH, 16 * G4), ml_dtypes.bfloat16)
    wh_bf = np.zeros((128, 4 * G4), ml_dtypes.bfloat16)
    for u in range(4):
        Wp = Ws[u][:, perm]  # [400, 400]
        bp = bs[u][perm]
        for k in range(3):
            blk = w_all[:, (u * 4 + k) * G4 : (u * 4 + k + 1) * G4]
            blk[0:100] = Wp[k * 100 : (k + 1) * 100]
        bias_row = bp.copy()
        bias_row[100:200] += FORGET_BIAS
        w_all[100, (u * 4 + 2) * G4 : (u * 4 + 3) * G4] = bias_row
        wh_bf[0:H, u * G4 : (u + 1) * G4] = Wp[300:400]
    return w_all, wh_bf


def kernel(premises, hypotheses, Wp_fw, bp_fw, Wp_bw, bp_bw, Wh_fw, bh_fw, Wh_bw, bh_bw):
    from concourse.bass_utils import run_bass_kernel_spmd

    if "nc" not in _CACHE:
        _CACHE["nc"] = _build_program()
    nc = _CACHE["nc"]

    w_all, wh_bf = _prep_weights(
        [Wp_fw, Wp_bw, Wh_fw, Wh_bw], [bp_fw, bp_bw, bh_fw, bh_bw]
    )
    import ml_dtypes

    ident = np.eye(128, dtype=ml_dtypes.bfloat16)

    in_maps = []
    for c in range(NCORES):
        sl = slice(c * BL, (c + 1) * BL)
        in_maps.append(
            {
                "xt_p": _prep_xt(np.asarray(premises[sl], np.float32)),
                "xt_h": _prep_xt(np.asarray(hypotheses[sl], np.float32)),
                "w_all": w_all,
                "wh_bf": wh_bf,
                "ident": ident,
            }
        )

    res = run_bass_kernel_spmd(nc, in_maps, core_ids=list(range(NCORES)))
    out = np.concatenate([r["out"] for r in res.results], axis=0)
    # columns are (c_pf, c_pb, c_hf, c_hb) in run order already
    return out
